# revision 24
# baseline (speedup 1.0000x reference)
"""GNN message-passing kernel for Trainium2 (8 NeuronCores, SPMD).

Computes out[D] = mean_n relu(segment_sum(val * (feat @ W.T + b)[src], dst))
reformulated as:
    agg[n]  = sum_{e: dst=n} val[e] * feature[src[e]]      (dma_gather + PE one-hot scatter)
    s[n]    = sum_{e: dst=n} val[e]                        (host-precomputed)
    z[n]    = agg[n] @ W.T + s[n] * b                      (PE, bias as K=1 rank-1 matmul)
    out     = sum_n relu(z[n]) / N                         (PE ones-reduce, host-combined)

Sharding: edges partitioned by destination node across 8 cores. Each core owns
N/8 destination nodes, split into groups of 128. Source-feature rows are
fetched with dma_gather (int16 indices => the node table is processed in
windows of 32768 rows). Per (window, group) bucket, 128-edge blocks are
scattered into a PSUM accumulator via one-hot matmuls; window partials
accumulate into an SBUF agg array. A per-group tail applies W/bias/relu and
reduces into a [1, D] partial that the host sums across cores.
"""

import contextlib
import math
import sys

import numpy as np

for _p in ("/opt/trn_rl_repo",):
    if _p not in sys.path:
        sys.path.insert(0, _p)

import concourse.bacc as bacc
import concourse.mybir as mybir
import concourse.tile as tile
from concourse.bass_utils import run_bass_kernel_spmd

P = 128
N_CORES = 8
WS = 32768  # gather window rows (int16 index reach)

F32 = None  # set at import below
try:
    F32 = mybir.dt.float32
except Exception:  # pragma: no cover
    pass


def _plan(N, E, edge_src, edge_dst, edge_val):
    """Host-side layout planning. Returns per-core input arrays + the
    compile-time block structure (shared across cores)."""
    nodes_per_core = (N + N_CORES - 1) // N_CORES
    n_groups = (nodes_per_core + P - 1) // P
    n_win = (N + WS - 1) // WS

    core = edge_dst // nodes_per_core
    local = edge_dst - core * nodes_per_core
    g = local // P
    ld = local % P
    w = edge_src // WS
    src_rel = (edge_src - w * WS).astype(np.int64)

    # bucket = (core, w, g)
    bkey = (core * n_win + w) * n_groups + g
    n_buckets = N_CORES * n_win * n_groups
    counts = np.bincount(bkey, minlength=n_buckets).reshape(N_CORES, n_win, n_groups)
    nb = np.ceil(counts.max(axis=0) / P).astype(np.int64)  # [n_win, n_groups]

    # window processing order: largest edge-count window last
    wtot = counts.sum(axis=(0, 2))
    worder = list(np.argsort(wtot, kind="stable"))

    # window-local block starts per bucket, window sizes
    b0 = np.zeros((n_win, n_groups), np.int64)
    NW = np.zeros(n_win, np.int64)
    for wi in range(n_win):
        b0[wi] = np.cumsum(nb[wi]) - nb[wi]
        NW[wi] = nb[wi].sum()
    woff = np.zeros(n_win, np.int64)  # global block offset, in processing order
    acc = 0
    for wv in worder:
        woff[wv] = acc
        acc += NW[wv]
    B = int(acc)

    # per-edge placement
    order = np.argsort(bkey, kind="stable")
    flat_counts = counts.reshape(-1)
    bucket_start = np.cumsum(flat_counts) - flat_counts
    rank = np.arange(E, dtype=np.int64) - bucket_start[bkey[order]]
    w_s = w[order]
    g_s = g[order]
    c_s = core[order]
    ipos = b0[w_s, g_s] * P + rank  # window-local gather position
    lane = ipos % P
    wblk = ipos // P
    gblk = woff[w_s] + wblk

    ldv = np.zeros((N_CORES, P, B), np.float32)
    valv = np.zeros((N_CORES, P, B), np.float32)
    ldv[c_s, lane, gblk] = ld[order].astype(np.float32)
    valv[c_s, lane, gblk] = edge_val[order]

    idx_wins = []
    for wv in range(n_win):
        iw = np.zeros((N_CORES, 16, int(NW[wv]) * 8), np.int16)
        idx_wins.append(iw)
    sel_w = [w_s == wv for wv in range(n_win)]
    for wv in range(n_win):
        m = sel_w[wv]
        ip = ipos[m]
        idx_wins[wv][c_s[m], ip % 16, ip // 16] = src_rel[order][m].astype(np.int16)
    idx_full = [np.tile(iw, (1, 8, 1)) for iw in idx_wins]  # [NC, 128, NW*8]

    skey = (core * n_groups + g) * P + ld
    s = np.bincount(
        skey, weights=edge_val.astype(np.float64), minlength=N_CORES * n_groups * P
    ).astype(np.float32)
    s = s.reshape(N_CORES, 1, n_groups * P)

    # compile-time structure: per window (processing order) bucket list
    structure = []
    for wv in worder:
        buckets = [(int(gg), int(nb[wv, gg])) for gg in range(n_groups) if nb[wv, gg] > 0]
        structure.append(
            {
                "w": int(wv),
                "row0": int(wv * WS),
                "rows": int(min(WS, N - wv * WS)),
                "NW": int(NW[wv]),
                "woff": int(woff[wv]),
                "buckets": buckets,
            }
        )
    return {
        "nodes_per_core": nodes_per_core,
        "n_groups": n_groups,
        "n_win": n_win,
        "B": B,
        "structure": structure,
        "ldv": ldv,
        "valv": valv,
        "idx": idx_full,
        "s": s,
    }


def _build_program(N, D, plan, dt, oh_pool_frac=0.0, CH=8, reps=1, ablate=()):
    f32 = mybir.dt.float32
    n_groups = plan["n_groups"]
    B = plan["B"]
    structure = plan["structure"]
    nc = bacc.Bacc(
        "TRN2",
        target_bir_lowering=False,
        debug=False,
        num_devices=N_CORES,
        num_swdge_queues=4,
        dynamic_dma_scratch_size=32768,
    )

    feature_t = nc.dram_tensor("feature", [N, D], dt, kind="ExternalInput")
    ld_t = nc.dram_tensor("ld", [P, B], f32, kind="ExternalInput")
    val_t = nc.dram_tensor("val", [P, B], f32, kind="ExternalInput")
    s_t = nc.dram_tensor("s", [1, n_groups * P], f32, kind="ExternalInput")
    wt_t = nc.dram_tensor("wt", [D, D], f32, kind="ExternalInput")
    brow_t = nc.dram_tensor("brow", [1, D], f32, kind="ExternalInput")
    iota_t = nc.dram_tensor("iota", [P, P], dt, kind="ExternalInput")
    ident_t = nc.dram_tensor("ident", [P, P], f32, kind="ExternalInput")
    ones_t = nc.dram_tensor("ones", [P, 1], f32, kind="ExternalInput")
    idx_ts = [
        nc.dram_tensor(f"idxw{st['w']}", [P, st["NW"] * 8], mybir.dt.int16,
                       kind="ExternalInput")
        for st in structure
    ]
    out_t = nc.dram_tensor("out", [1, D], f32, kind="ExternalOutput")

    # first/last window (processing order) in which each group has blocks
    last_win_of_g = {}
    first_win_of_g = {}
    seen_g = set()
    for si, st in enumerate(structure):
        for gg, _ in st["buckets"]:
            last_win_of_g[gg] = si
            if gg not in first_win_of_g:
                first_win_of_g[gg] = si
            seen_g.add(gg)

    with tile.TileContext(nc) as tc:
        with (
            tc.tile_pool(name="const", bufs=1) as constp,
            tc.tile_pool(name="idxp", bufs=2) as idxp,
            tc.tile_pool(name="msg", bufs=8) as msgp,
            tc.tile_pool(name="oh", bufs=8) as ohp,
            tc.tile_pool(name="gsb", bufs=3) as gsbp,
            tc.tile_pool(name="agg", bufs=3, space="PSUM") as aggp,
            tc.tile_pool(name="tr", bufs=2, space="PSUM") as trp,
            tc.tile_pool(name="z", bufs=2, space="PSUM") as zp,
            tc.tile_pool(name="acc", bufs=1, space="PSUM") as accp,
        ):
            ld_sb = constp.tile([P, B], f32)
            nc.sync.dma_start(ld_sb[:], ld_t[:])
            val_sb = constp.tile([P, B], f32)
            nc.sync.dma_start(val_sb[:], val_t[:])
            s_sb = constp.tile([1, n_groups * P], f32)
            nc.sync.dma_start(s_sb[:], s_t[:])
            wt_sb = constp.tile([D, D], f32)
            nc.sync.dma_start(wt_sb[:], wt_t[:])
            brow_sb = constp.tile([1, D], f32)
            nc.sync.dma_start(brow_sb[:], brow_t[:])
            iota_sb = constp.tile([P, P], dt)
            nc.sync.dma_start(iota_sb[:], iota_t[:])
            ident_sb = constp.tile([P, P], f32)
            nc.sync.dma_start(ident_sb[:], ident_t[:])
            ones_sb = constp.tile([P, 1], f32)
            nc.sync.dma_start(ones_sb[:], ones_t[:])

            agg_acc = constp.tile([P, n_groups * P], f32)
            out_acc = accp.tile([1, D], f32)

            msg0 = None
            if "gather" in ablate:
                msg0 = constp.tile([P, CH, D], dt)
                nc.vector.memset(msg0[:], 0.0)

            Copy = mybir.ActivationFunctionType.Copy
            Relu = mybir.ActivationFunctionType.Relu

            n_tails = [0]

            def tail(gg):
                gsl = slice(gg * P, (gg + 1) * P)
                aggT_ps = trp.tile([P, D], f32)
                nc.tensor.transpose(
                    out=aggT_ps[:], in_=agg_acc[:, gsl], identity=ident_sb[:]
                )
                aggT_sb = gsbp.tile([P, D], f32, tag="aggT_sb")
                nc.scalar.activation(out=aggT_sb[:], in_=aggT_ps[:], func=Copy)
                z_ps = zp.tile([P, D], f32)
                nc.tensor.matmul(
                    out=z_ps[:], lhsT=aggT_sb[:], rhs=wt_sb[:], start=True, stop=False
                )
                nc.tensor.matmul(
                    out=z_ps[:],
                    lhsT=s_sb[0:1, gsl],
                    rhs=brow_sb[:],
                    start=False,
                    stop=True,
                )
                relu_sb = gsbp.tile([P, D], f32, tag="relu")
                nc.scalar.activation(out=relu_sb[:], in_=z_ps[:], func=Relu)
                gi = n_tails[0]
                n_tails[0] += 1
                nc.tensor.matmul(
                    out=out_acc[0:1, :],
                    lhsT=ones_sb[:],
                    rhs=relu_sb[:],
                    start=(gi == 0),
                    stop=(gi == n_groups - 1),
                )

            oh_count = [0]
            gq_count = [0]

            rep_ctx = (
                tc.For_i(0, reps, name="rep") if reps > 1
                else contextlib.nullcontext()
            )

            def make_onehot(bb):
                oh = ohp.tile([P, P], dt)
                eng = nc.vector
                if oh_pool_frac > 0:
                    oh_count[0] += 1
                    if (oh_count[0] % 1000) < oh_pool_frac * 1000:
                        eng = nc.gpsimd
                eng.tensor_scalar(
                    oh[:],
                    iota_sb[:],
                    ld_sb[:, bb : bb + 1],
                    val_sb[:, bb : bb + 1],
                    mybir.AluOpType.is_equal,
                    mybir.AluOpType.mult,
                )
                return oh

            with rep_ctx:
                for si, st in enumerate(structure):
                    NW = st["NW"]
                    if NW == 0:
                        continue
                    idx_sb = idxp.tile([P, NW * 8], mybir.dt.int16, tag="idx")
                    nc.sync.dma_start(idx_sb[:], idx_ts[si][:])
                    fwin = feature_t[st["row0"] : st["row0"] + st["rows"], :]
                    msg = None
                    wb_cursor = 0
                    for gg, nbk in st["buckets"]:
                        agg_ps = aggp.tile([P, D], f32)
                        for j in range(nbk):
                            wb = wb_cursor + j
                            c, r = divmod(wb, CH)
                            if r == 0:
                                cw = min(CH, NW - c * CH)
                                if "gather" in ablate:
                                    msg = msg0
                                else:
                                    msg = msgp.tile([P, CH, D], dt, tag="msg")
                                    nc.gpsimd.dma_gather(
                                        out_ap=msg[:, :cw, :],
                                        in_ap=fwin,
                                        idxs_ap=idx_sb[:, c * CH * 8 : (c * CH + cw) * 8],
                                        num_idxs=cw * P,
                                        num_idxs_reg=cw * P,
                                        elem_size=D,
                                        queue_num=gq_count[0] % 4,
                                        single_packet=False,
                                    )
                                gq_count[0] += 1
                            bb = st["woff"] + wb
                            oh = make_onehot(bb)
                            nc.tensor.matmul(
                                out=agg_ps[:],
                                lhsT=oh[:],
                                rhs=msg[:, r, :],
                                start=(j == 0),
                                stop=(j == nbk - 1),
                            )
                        wb_cursor += nbk
                        gsl = slice(gg * P, (gg + 1) * P)
                        if si == first_win_of_g[gg]:
                            nc.scalar.activation(
                                out=agg_acc[:, gsl], in_=agg_ps[:], func=Copy
                            )
                        else:
                            nc.vector.tensor_tensor(
                                out=agg_acc[:, gsl],
                                in0=agg_acc[:, gsl],
                                in1=agg_ps[:],
                                op=mybir.AluOpType.add,
                            )
                        if last_win_of_g[gg] == si:
                            tail(gg)

                # groups with no edges at all: agg is zero -> z = 0 -> relu 0.
                # still must contribute to the reduce chain count; memset + tail.
                for gg in range(n_groups):
                    if gg not in seen_g:
                        nc.vector.memset(agg_acc[:, gg * P : (gg + 1) * P], 0.0)
                        tail(gg)

                res_sb = constp.tile([1, D], f32)
                nc.vector.tensor_copy(res_sb[:], out_acc[0:1, :])
                nc.sync.dma_start(out_t[:], res_sb[:])

    nc.compile()
    return nc


def prepare(feature, edge_src, edge_dst, edge_val, W, b, reps=1,
            use_bf16=True, CH=16):
    """Build the Bass program + per-core input maps. Returns (nc, in_maps, N)."""
    N, D = feature.shape
    E = edge_src.shape[0]
    assert D == P

    feature = np.ascontiguousarray(feature, dtype=np.float32)
    edge_src = np.asarray(edge_src, dtype=np.int64)
    edge_dst = np.asarray(edge_dst, dtype=np.int64)
    edge_val = np.asarray(edge_val, dtype=np.float32)

    plan = _plan(N, E, edge_src, edge_dst, edge_val)

    import os as _os
    ablate = tuple(_os.environ.get("ABLATE", "").split(",")) if _os.environ.get("ABLATE") else ()
    dt = mybir.dt.bfloat16 if use_bf16 else mybir.dt.float32
    nc = _build_program(N, D, plan, dt, CH=CH, reps=reps, ablate=ablate)

    npdt = mybir.dt.np(dt)
    wt = np.ascontiguousarray(W.astype(np.float32).T)
    brow = np.ascontiguousarray(b.astype(np.float32).reshape(1, D))
    iota = np.tile(np.arange(P, dtype=np.float32), (P, 1)).astype(npdt)
    ident = np.eye(P, dtype=np.float32)
    ones = np.ones((P, 1), dtype=np.float32)
    feature_c = feature.astype(npdt)

    in_maps = []
    for c in range(N_CORES):
        m = {
            "feature": feature_c,
            "ld": plan["ldv"][c],
            "val": plan["valv"][c],
            "s": plan["s"][c],
            "wt": wt,
            "brow": brow,
            "iota": iota,
            "ident": ident,
            "ones": ones,
        }
        for st in plan["structure"]:
            m[f"idxw{st['w']}"] = plan["idx"][st["w"]][c]
        in_maps.append(m)

    return nc, in_maps, N


def combine(results, N):
    parts = np.stack([results[c]["out"][0] for c in range(N_CORES)])
    return (parts.sum(axis=0, dtype=np.float64) / N).astype(np.float32)


def kernel(feature, edge_src, edge_dst, edge_val, W, b):
    nc, in_maps, N = prepare(feature, edge_src, edge_dst, edge_val, W, b)
    res = run_bass_kernel_spmd(nc, in_maps, core_ids=list(range(N_CORES)))
    kernel.last = res  # for test.py profiling; harmless in harness
    return combine(res.results, N)



# revision 25
# speedup vs baseline: 1.3664x; 1.3664x over previous
"""GNN message-passing kernel for Trainium2 (8 NeuronCores, SPMD).

Computes out[D] = mean_n relu(segment_sum(val * (feat @ W.T + b)[src], dst))
reformulated as:
    agg[n]  = sum_{e: dst=n} val[e] * feature[src[e]]      (dma_gather + PE one-hot scatter)
    s[n]    = sum_{e: dst=n} val[e]                        (host-precomputed)
    z[n]    = agg[n] @ W.T + s[n] * b                      (PE, bias as K=1 rank-1 matmul)
    out     = sum_n relu(z[n]) / N                         (PE ones-reduce, host-combined)

Sharding: edges partitioned by destination node across 8 cores. Each core owns
N/8 destination nodes, split into groups of 128. Source-feature rows are
fetched with dma_gather (int16 indices => the node table is processed in
windows of 32768 rows). Per (window, group) bucket, 128-edge blocks are
scattered into a PSUM accumulator via one-hot matmuls; window partials
accumulate into an SBUF agg array. A per-group tail applies W/bias/relu and
reduces into a [1, D] partial that the host sums across cores.
"""

import contextlib
import math
import sys

import numpy as np

for _p in ("/opt/trn_rl_repo",):
    if _p not in sys.path:
        sys.path.insert(0, _p)

import concourse.bacc as bacc
import concourse.mybir as mybir
import concourse.tile as tile
from concourse.bass_utils import run_bass_kernel_spmd

P = 128
N_CORES = 8
WS = 32768  # gather window rows (int16 index reach)

F32 = None  # set at import below
try:
    F32 = mybir.dt.float32
except Exception:  # pragma: no cover
    pass


def _plan(N, E, edge_src, edge_dst, edge_val):
    """Host-side layout planning. Returns per-core input arrays + the
    compile-time block structure (shared across cores)."""
    nodes_per_core = (N + N_CORES - 1) // N_CORES
    n_groups = (nodes_per_core + P - 1) // P
    n_win = (N + WS - 1) // WS

    core = edge_dst // nodes_per_core
    local = edge_dst - core * nodes_per_core
    g = local // P
    ld = local % P
    w = edge_src // WS
    src_rel = (edge_src - w * WS).astype(np.int64)

    # bucket = (core, w, g)
    bkey = (core * n_win + w) * n_groups + g
    n_buckets = N_CORES * n_win * n_groups
    counts = np.bincount(bkey, minlength=n_buckets).reshape(N_CORES, n_win, n_groups)
    nb = np.ceil(counts.max(axis=0) / P).astype(np.int64)  # [n_win, n_groups]

    # window processing order: largest edge-count window last
    wtot = counts.sum(axis=(0, 2))
    worder = list(np.argsort(wtot, kind="stable"))

    # window-local block starts per bucket, window sizes
    b0 = np.zeros((n_win, n_groups), np.int64)
    NW = np.zeros(n_win, np.int64)
    for wi in range(n_win):
        b0[wi] = np.cumsum(nb[wi]) - nb[wi]
        NW[wi] = nb[wi].sum()
    woff = np.zeros(n_win, np.int64)  # global block offset, in processing order
    acc = 0
    for wv in worder:
        woff[wv] = acc
        acc += NW[wv]
    B = int(acc)

    # per-edge placement
    order = np.argsort(bkey, kind="stable")
    flat_counts = counts.reshape(-1)
    bucket_start = np.cumsum(flat_counts) - flat_counts
    rank = np.arange(E, dtype=np.int64) - bucket_start[bkey[order]]
    w_s = w[order]
    g_s = g[order]
    c_s = core[order]
    ipos = b0[w_s, g_s] * P + rank  # window-local gather position
    lane = ipos % P
    wblk = ipos // P
    gblk = woff[w_s] + wblk

    ldv = np.zeros((N_CORES, P, B), np.float32)
    valv = np.zeros((N_CORES, P, B), np.float32)
    ldv[c_s, lane, gblk] = ld[order].astype(np.float32)
    valv[c_s, lane, gblk] = edge_val[order]

    idx_wins = []
    for wv in range(n_win):
        iw = np.zeros((N_CORES, 16, int(NW[wv]) * 8), np.int16)
        idx_wins.append(iw)
    sel_w = [w_s == wv for wv in range(n_win)]
    for wv in range(n_win):
        m = sel_w[wv]
        ip = ipos[m]
        idx_wins[wv][c_s[m], ip % 16, ip // 16] = src_rel[order][m].astype(np.int16)
    idx_full = [np.tile(iw, (1, 8, 1)) for iw in idx_wins]  # [NC, 128, NW*8]

    skey = (core * n_groups + g) * P + ld
    s = np.bincount(
        skey, weights=edge_val.astype(np.float64), minlength=N_CORES * n_groups * P
    ).astype(np.float32)
    s = s.reshape(N_CORES, 1, n_groups * P)

    # compile-time structure: per window (processing order) bucket list
    structure = []
    for wv in worder:
        buckets = [(int(gg), int(nb[wv, gg])) for gg in range(n_groups) if nb[wv, gg] > 0]
        structure.append(
            {
                "w": int(wv),
                "row0": int(wv * WS),
                "rows": int(min(WS, N - wv * WS)),
                "NW": int(NW[wv]),
                "woff": int(woff[wv]),
                "buckets": buckets,
            }
        )
    return {
        "nodes_per_core": nodes_per_core,
        "n_groups": n_groups,
        "n_win": n_win,
        "B": B,
        "structure": structure,
        "ldv": ldv,
        "valv": valv,
        "idx": idx_full,
        "s": s,
    }


def _build_program(N, D, plan, dt, oh_pool_frac=0.0, CH=8, reps=1, ablate=()):
    f32 = mybir.dt.float32
    n_groups = plan["n_groups"]
    B = plan["B"]
    structure = plan["structure"]
    nc = bacc.Bacc(
        "TRN2",
        target_bir_lowering=False,
        debug=False,
        num_devices=N_CORES,
        num_swdge_queues=4,
    )

    feature_t = nc.dram_tensor("feature", [N, D], dt, kind="ExternalInput")
    ld_t = nc.dram_tensor("ld", [P, B], f32, kind="ExternalInput")
    val_t = nc.dram_tensor("val", [P, B], f32, kind="ExternalInput")
    s_t = nc.dram_tensor("s", [1, n_groups * P], f32, kind="ExternalInput")
    wt_t = nc.dram_tensor("wt", [D, D], f32, kind="ExternalInput")
    brow_t = nc.dram_tensor("brow", [1, D], f32, kind="ExternalInput")
    iota_t = nc.dram_tensor("iota", [P, P], dt, kind="ExternalInput")
    ident_t = nc.dram_tensor("ident", [P, P], f32, kind="ExternalInput")
    ones_t = nc.dram_tensor("ones", [P, 1], f32, kind="ExternalInput")
    idx_ts = [
        nc.dram_tensor(f"idxw{st['w']}", [P, st["NW"] * 8], mybir.dt.int16,
                       kind="ExternalInput")
        for st in structure
    ]
    out_t = nc.dram_tensor("out", [1, D], f32, kind="ExternalOutput")

    # first/last window (processing order) in which each group has blocks
    last_win_of_g = {}
    first_win_of_g = {}
    seen_g = set()
    for si, st in enumerate(structure):
        for gg, _ in st["buckets"]:
            last_win_of_g[gg] = si
            if gg not in first_win_of_g:
                first_win_of_g[gg] = si
            seen_g.add(gg)

    with tile.TileContext(nc) as tc:
        with (
            tc.tile_pool(name="const", bufs=1) as constp,
            tc.tile_pool(name="idxp", bufs=2) as idxp,
            tc.tile_pool(name="msg", bufs=8) as msgp,
            tc.tile_pool(name="oh", bufs=8) as ohp,
            tc.tile_pool(name="gsb", bufs=3) as gsbp,
            tc.tile_pool(name="agg", bufs=2, space="PSUM") as aggp,
            tc.tile_pool(name="tr", bufs=2, space="PSUM") as trp,
            tc.tile_pool(name="z", bufs=2, space="PSUM") as zp,
            tc.tile_pool(name="acc", bufs=1, space="PSUM") as accp,
        ):
            ld_sb = constp.tile([P, B], f32)
            nc.sync.dma_start(ld_sb[:], ld_t[:])
            val_sb = constp.tile([P, B], f32)
            nc.sync.dma_start(val_sb[:], val_t[:])
            s_sb = constp.tile([1, n_groups * P], f32)
            nc.sync.dma_start(s_sb[:], s_t[:])
            wt_sb = constp.tile([D, D], f32)
            nc.sync.dma_start(wt_sb[:], wt_t[:])
            brow_sb = constp.tile([1, D], f32)
            nc.sync.dma_start(brow_sb[:], brow_t[:])
            iota_sb = constp.tile([P, P], dt)
            nc.sync.dma_start(iota_sb[:], iota_t[:])
            ident_sb = constp.tile([P, P], f32)
            nc.sync.dma_start(ident_sb[:], ident_t[:])
            ones_sb = constp.tile([P, 1], f32)
            nc.sync.dma_start(ones_sb[:], ones_t[:])

            agg_acc = constp.tile([P, n_groups * P], f32)
            out_acc = accp.tile([1, D], f32)

            msg0 = None
            if "gather" in ablate:
                msg0 = constp.tile([P, CH, D], dt)
                nc.vector.memset(msg0[:], 0.0)

            Copy = mybir.ActivationFunctionType.Copy
            Relu = mybir.ActivationFunctionType.Relu

            n_tails = [0]

            def tail(gg):
                gsl = slice(gg * P, (gg + 1) * P)
                aggT_ps = trp.tile([P, D], f32)
                nc.tensor.transpose(
                    out=aggT_ps[:], in_=agg_acc[:, gsl], identity=ident_sb[:]
                )
                aggT_sb = gsbp.tile([P, D], f32, tag="aggT_sb")
                nc.scalar.activation(out=aggT_sb[:], in_=aggT_ps[:], func=Copy)
                z_ps = zp.tile([P, D], f32)
                nc.tensor.matmul(
                    out=z_ps[:], lhsT=aggT_sb[:], rhs=wt_sb[:], start=True, stop=False
                )
                nc.tensor.matmul(
                    out=z_ps[:],
                    lhsT=s_sb[0:1, gsl],
                    rhs=brow_sb[:],
                    start=False,
                    stop=True,
                )
                relu_sb = gsbp.tile([P, D], f32, tag="relu")
                nc.scalar.activation(out=relu_sb[:], in_=z_ps[:], func=Relu)
                gi = n_tails[0]
                n_tails[0] += 1
                nc.tensor.matmul(
                    out=out_acc[0:1, :],
                    lhsT=ones_sb[:],
                    rhs=relu_sb[:],
                    start=(gi == 0),
                    stop=(gi == n_groups - 1),
                )

            oh_count = [0]
            gq_count = [0]

            rep_ctx = (
                tc.For_i(0, reps, name="rep") if reps > 1
                else contextlib.nullcontext()
            )

            def make_onehot(bb):
                oh = ohp.tile([P, P], dt)
                eng = nc.vector
                if oh_pool_frac > 0:
                    oh_count[0] += 1
                    if (oh_count[0] % 1000) < oh_pool_frac * 1000:
                        eng = nc.gpsimd
                eng.tensor_scalar(
                    oh[:],
                    iota_sb[:],
                    ld_sb[:, bb : bb + 1],
                    val_sb[:, bb : bb + 1],
                    mybir.AluOpType.is_equal,
                    mybir.AluOpType.mult,
                )
                return oh

            with rep_ctx:
                for si, st in enumerate(structure):
                    NW = st["NW"]
                    if NW == 0:
                        continue
                    idx_sb = idxp.tile([P, NW * 8], mybir.dt.int16, tag="idx")
                    nc.sync.dma_start(idx_sb[:], idx_ts[si][:])
                    fwin = feature_t[st["row0"] : st["row0"] + st["rows"], :]
                    msg = None
                    wb_cursor = 0
                    for gg, nbk in st["buckets"]:
                        gsl = slice(gg * P, (gg + 1) * P)
                        agg_ps = aggp.tile([P, D], f32)
                        carry = si != first_win_of_g[gg]
                        if carry:
                            nc.tensor.matmul(
                                out=agg_ps[:],
                                lhsT=ident_sb[:],
                                rhs=agg_acc[:, gsl],
                                start=True,
                                stop=False,
                            )
                        for j in range(nbk):
                            wb = wb_cursor + j
                            c, r = divmod(wb, CH)
                            if r == 0:
                                cw = min(CH, NW - c * CH)
                                if "gather" in ablate:
                                    msg = msg0
                                else:
                                    msg = msgp.tile([P, CH, D], dt, tag="msg")
                                    nc.gpsimd.dma_gather(
                                        out_ap=msg[:, :cw, :],
                                        in_ap=fwin,
                                        idxs_ap=idx_sb[:, c * CH * 8 : (c * CH + cw) * 8],
                                        num_idxs=cw * P,
                                        num_idxs_reg=cw * P,
                                        elem_size=D,
                                        queue_num=gq_count[0] % 4,
                                        single_packet=False,
                                    )
                                gq_count[0] += 1
                            bb = st["woff"] + wb
                            oh = make_onehot(bb)
                            nc.tensor.matmul(
                                out=agg_ps[:],
                                lhsT=oh[:],
                                rhs=msg[:, r, :],
                                start=(j == 0 and not carry),
                                stop=(j == nbk - 1),
                            )
                        wb_cursor += nbk
                        nc.scalar.activation(
                            out=agg_acc[:, gsl], in_=agg_ps[:], func=Copy
                        )
                        if last_win_of_g[gg] == si:
                            tail(gg)

                # groups with no edges at all: agg is zero -> z = 0 -> relu 0.
                # still must contribute to the reduce chain count; memset + tail.
                for gg in range(n_groups):
                    if gg not in seen_g:
                        nc.vector.memset(agg_acc[:, gg * P : (gg + 1) * P], 0.0)
                        tail(gg)

                res_sb = constp.tile([1, D], f32)
                nc.vector.tensor_copy(res_sb[:], out_acc[0:1, :])
                nc.sync.dma_start(out_t[:], res_sb[:])

    nc.compile()
    return nc


def prepare(feature, edge_src, edge_dst, edge_val, W, b, reps=1,
            use_bf16=True, CH=8):
    """Build the Bass program + per-core input maps. Returns (nc, in_maps, N)."""
    N, D = feature.shape
    E = edge_src.shape[0]
    assert D == P

    feature = np.ascontiguousarray(feature, dtype=np.float32)
    edge_src = np.asarray(edge_src, dtype=np.int64)
    edge_dst = np.asarray(edge_dst, dtype=np.int64)
    edge_val = np.asarray(edge_val, dtype=np.float32)

    plan = _plan(N, E, edge_src, edge_dst, edge_val)

    import os as _os
    ablate = tuple(_os.environ.get("ABLATE", "").split(",")) if _os.environ.get("ABLATE") else ()
    dt = mybir.dt.bfloat16 if use_bf16 else mybir.dt.float32
    nc = _build_program(N, D, plan, dt, CH=CH, reps=reps, ablate=ablate)

    npdt = mybir.dt.np(dt)
    wt = np.ascontiguousarray(W.astype(np.float32).T)
    brow = np.ascontiguousarray(b.astype(np.float32).reshape(1, D))
    iota = np.tile(np.arange(P, dtype=np.float32), (P, 1)).astype(npdt)
    ident = np.eye(P, dtype=np.float32)
    ones = np.ones((P, 1), dtype=np.float32)
    feature_c = feature.astype(npdt)

    in_maps = []
    for c in range(N_CORES):
        m = {
            "feature": feature_c,
            "ld": plan["ldv"][c],
            "val": plan["valv"][c],
            "s": plan["s"][c],
            "wt": wt,
            "brow": brow,
            "iota": iota,
            "ident": ident,
            "ones": ones,
        }
        for st in plan["structure"]:
            m[f"idxw{st['w']}"] = plan["idx"][st["w"]][c]
        in_maps.append(m)

    return nc, in_maps, N


def combine(results, N):
    parts = np.stack([results[c]["out"][0] for c in range(N_CORES)])
    return (parts.sum(axis=0, dtype=np.float64) / N).astype(np.float32)


def kernel(feature, edge_src, edge_dst, edge_val, W, b):
    nc, in_maps, N = prepare(feature, edge_src, edge_dst, edge_val, W, b)
    res = run_bass_kernel_spmd(nc, in_maps, core_ids=list(range(N_CORES)))
    kernel.last = res  # for test.py profiling; harmless in harness
    return combine(res.results, N)



# revision 26
# speedup vs baseline: 1.3735x; 1.0052x over previous
"""GNN message-passing kernel for Trainium2 (8 NeuronCores, SPMD).

Computes out[D] = mean_n relu(segment_sum(val * (feat @ W.T + b)[src], dst))
reformulated as:
    agg[n]  = sum_{e: dst=n} val[e] * feature[src[e]]      (dma_gather + PE one-hot scatter)
    s[n]    = sum_{e: dst=n} val[e]                        (host-precomputed)
    z[n]    = agg[n] @ W.T + s[n] * b                      (PE, bias as K=1 rank-1 matmul)
    out     = sum_n relu(z[n]) / N                         (PE ones-reduce, host-combined)

Sharding: edges partitioned by destination node across 8 cores. Each core owns
N/8 destination nodes, split into groups of 128. Source-feature rows are
fetched with dma_gather (int16 indices => the node table is processed in
windows of 32768 rows). Per (window, group) bucket, 128-edge blocks are
scattered into a PSUM accumulator via one-hot matmuls; window partials
accumulate into an SBUF agg array. A per-group tail applies W/bias/relu and
reduces into a [1, D] partial that the host sums across cores.
"""

import contextlib
import math
import sys

import numpy as np

for _p in ("/opt/trn_rl_repo",):
    if _p not in sys.path:
        sys.path.insert(0, _p)

import concourse.bacc as bacc
import concourse.mybir as mybir
import concourse.tile as tile
from concourse.bass_utils import run_bass_kernel_spmd

P = 128
N_CORES = 8
WS = 32768  # gather window rows (int16 index reach)

F32 = None  # set at import below
try:
    F32 = mybir.dt.float32
except Exception:  # pragma: no cover
    pass


def _plan(N, E, edge_src, edge_dst, edge_val):
    """Host-side layout planning. Returns per-core input arrays + the
    compile-time block structure (shared across cores)."""
    nodes_per_core = (N + N_CORES - 1) // N_CORES
    n_groups = (nodes_per_core + P - 1) // P
    n_win = (N + WS - 1) // WS

    core = edge_dst // nodes_per_core
    local = edge_dst - core * nodes_per_core
    g = local // P
    ld = local % P
    w = edge_src // WS
    src_rel = (edge_src - w * WS).astype(np.int64)

    # bucket = (core, w, g)
    bkey = (core * n_win + w) * n_groups + g
    n_buckets = N_CORES * n_win * n_groups
    counts = np.bincount(bkey, minlength=n_buckets).reshape(N_CORES, n_win, n_groups)
    nb = np.ceil(counts.max(axis=0) / P).astype(np.int64)  # [n_win, n_groups]

    # window processing order: largest edge-count window last
    wtot = counts.sum(axis=(0, 2))
    worder = list(np.argsort(wtot, kind="stable"))

    # window-local block starts per bucket, window sizes
    b0 = np.zeros((n_win, n_groups), np.int64)
    NW = np.zeros(n_win, np.int64)
    for wi in range(n_win):
        b0[wi] = np.cumsum(nb[wi]) - nb[wi]
        NW[wi] = nb[wi].sum()
    woff = np.zeros(n_win, np.int64)  # global block offset, in processing order
    acc = 0
    for wv in worder:
        woff[wv] = acc
        acc += NW[wv]
    B = int(acc)

    # per-edge placement
    order = np.argsort(bkey, kind="stable")
    flat_counts = counts.reshape(-1)
    bucket_start = np.cumsum(flat_counts) - flat_counts
    rank = np.arange(E, dtype=np.int64) - bucket_start[bkey[order]]
    w_s = w[order]
    g_s = g[order]
    c_s = core[order]
    ipos = b0[w_s, g_s] * P + rank  # window-local gather position
    lane = ipos % P
    wblk = ipos // P
    gblk = woff[w_s] + wblk

    ldv = np.zeros((N_CORES, P, B), np.float32)
    valv = np.zeros((N_CORES, P, B), np.float32)
    ldv[c_s, lane, gblk] = ld[order].astype(np.float32)
    valv[c_s, lane, gblk] = edge_val[order]

    idx_wins = []
    for wv in range(n_win):
        iw = np.zeros((N_CORES, 16, int(NW[wv]) * 8), np.int16)
        idx_wins.append(iw)
    sel_w = [w_s == wv for wv in range(n_win)]
    for wv in range(n_win):
        m = sel_w[wv]
        ip = ipos[m]
        idx_wins[wv][c_s[m], ip % 16, ip // 16] = src_rel[order][m].astype(np.int16)
    idx_full = [np.tile(iw, (1, 8, 1)) for iw in idx_wins]  # [NC, 128, NW*8]

    skey = (core * n_groups + g) * P + ld
    s = np.bincount(
        skey, weights=edge_val.astype(np.float64), minlength=N_CORES * n_groups * P
    ).astype(np.float32)
    s = s.reshape(N_CORES, 1, n_groups * P)

    # compile-time structure: per window (processing order) bucket list
    structure = []
    for wv in worder:
        buckets = [(int(gg), int(nb[wv, gg])) for gg in range(n_groups) if nb[wv, gg] > 0]
        structure.append(
            {
                "w": int(wv),
                "row0": int(wv * WS),
                "rows": int(min(WS, N - wv * WS)),
                "NW": int(NW[wv]),
                "woff": int(woff[wv]),
                "buckets": buckets,
            }
        )
    return {
        "nodes_per_core": nodes_per_core,
        "n_groups": n_groups,
        "n_win": n_win,
        "B": B,
        "structure": structure,
        "ldv": ldv,
        "valv": valv,
        "idx": idx_full,
        "s": s,
    }


def _build_program(N, D, plan, dt, oh_pool_frac=0.0, CH=8, reps=1, ablate=()):
    f32 = mybir.dt.float32
    n_groups = plan["n_groups"]
    B = plan["B"]
    structure = plan["structure"]
    nc = bacc.Bacc(
        "TRN2",
        target_bir_lowering=False,
        debug=False,
        num_devices=N_CORES,
        num_swdge_queues=4,
    )

    feature_t = nc.dram_tensor("feature", [N, D], dt, kind="ExternalInput")
    ld_t = nc.dram_tensor("ld", [P, B], f32, kind="ExternalInput")
    val_t = nc.dram_tensor("val", [P, B], f32, kind="ExternalInput")
    s_t = nc.dram_tensor("s", [1, n_groups * P], f32, kind="ExternalInput")
    wt_t = nc.dram_tensor("wt", [D, D], f32, kind="ExternalInput")
    brow_t = nc.dram_tensor("brow", [1, D], f32, kind="ExternalInput")
    iota_t = nc.dram_tensor("iota", [P, P], dt, kind="ExternalInput")
    ident_t = nc.dram_tensor("ident", [P, P], f32, kind="ExternalInput")
    ones_t = nc.dram_tensor("ones", [P, 1], f32, kind="ExternalInput")
    idx_ts = [
        nc.dram_tensor(f"idxw{st['w']}", [P, st["NW"] * 8], mybir.dt.int16,
                       kind="ExternalInput")
        for st in structure
    ]
    out_t = nc.dram_tensor("out", [1, D], f32, kind="ExternalOutput")

    # first/last window (processing order) in which each group has blocks
    last_win_of_g = {}
    first_win_of_g = {}
    seen_g = set()
    for si, st in enumerate(structure):
        for gg, _ in st["buckets"]:
            last_win_of_g[gg] = si
            if gg not in first_win_of_g:
                first_win_of_g[gg] = si
            seen_g.add(gg)

    with tile.TileContext(nc) as tc:
        with (
            tc.tile_pool(name="const", bufs=1) as constp,
            tc.tile_pool(name="idxp", bufs=2) as idxp,
            tc.tile_pool(name="msg", bufs=12) as msgp,
            tc.tile_pool(name="oh", bufs=12) as ohp,
            tc.tile_pool(name="gsb", bufs=3) as gsbp,
            tc.tile_pool(name="agg", bufs=2, space="PSUM") as aggp,
            tc.tile_pool(name="tr", bufs=2, space="PSUM") as trp,
            tc.tile_pool(name="z", bufs=2, space="PSUM") as zp,
            tc.tile_pool(name="acc", bufs=1, space="PSUM") as accp,
        ):
            ld_sb = constp.tile([P, B], f32)
            nc.sync.dma_start(ld_sb[:], ld_t[:])
            val_sb = constp.tile([P, B], f32)
            nc.sync.dma_start(val_sb[:], val_t[:])
            s_sb = constp.tile([1, n_groups * P], f32)
            nc.sync.dma_start(s_sb[:], s_t[:])
            wt_sb = constp.tile([D, D], f32)
            nc.sync.dma_start(wt_sb[:], wt_t[:])
            brow_sb = constp.tile([1, D], f32)
            nc.sync.dma_start(brow_sb[:], brow_t[:])
            iota_sb = constp.tile([P, P], dt)
            nc.sync.dma_start(iota_sb[:], iota_t[:])
            ident_sb = constp.tile([P, P], f32)
            nc.sync.dma_start(ident_sb[:], ident_t[:])
            ones_sb = constp.tile([P, 1], f32)
            nc.sync.dma_start(ones_sb[:], ones_t[:])

            agg_acc = constp.tile([P, n_groups * P], f32)
            out_acc = accp.tile([1, D], f32)

            msg0 = None
            if "gather" in ablate:
                msg0 = constp.tile([P, CH, D], dt)
                nc.vector.memset(msg0[:], 0.0)

            Copy = mybir.ActivationFunctionType.Copy
            Relu = mybir.ActivationFunctionType.Relu

            n_tails = [0]

            def tail(gg):
                gsl = slice(gg * P, (gg + 1) * P)
                aggT_ps = trp.tile([P, D], f32)
                nc.tensor.transpose(
                    out=aggT_ps[:], in_=agg_acc[:, gsl], identity=ident_sb[:]
                )
                aggT_sb = gsbp.tile([P, D], f32, tag="aggT_sb")
                nc.scalar.activation(out=aggT_sb[:], in_=aggT_ps[:], func=Copy)
                z_ps = zp.tile([P, D], f32)
                nc.tensor.matmul(
                    out=z_ps[:], lhsT=aggT_sb[:], rhs=wt_sb[:], start=True, stop=False
                )
                nc.tensor.matmul(
                    out=z_ps[:],
                    lhsT=s_sb[0:1, gsl],
                    rhs=brow_sb[:],
                    start=False,
                    stop=True,
                )
                relu_sb = gsbp.tile([P, D], f32, tag="relu")
                nc.scalar.activation(out=relu_sb[:], in_=z_ps[:], func=Relu)
                gi = n_tails[0]
                n_tails[0] += 1
                nc.tensor.matmul(
                    out=out_acc[0:1, :],
                    lhsT=ones_sb[:],
                    rhs=relu_sb[:],
                    start=(gi == 0),
                    stop=(gi == n_groups - 1),
                )

            oh_count = [0]
            gq_count = [0]

            rep_ctx = (
                tc.For_i(0, reps, name="rep") if reps > 1
                else contextlib.nullcontext()
            )

            def make_onehot(bb):
                oh = ohp.tile([P, P], dt)
                eng = nc.vector
                if oh_pool_frac > 0:
                    oh_count[0] += 1
                    if (oh_count[0] % 1000) < oh_pool_frac * 1000:
                        eng = nc.gpsimd
                eng.tensor_scalar(
                    oh[:],
                    iota_sb[:],
                    ld_sb[:, bb : bb + 1],
                    val_sb[:, bb : bb + 1],
                    mybir.AluOpType.is_equal,
                    mybir.AluOpType.mult,
                )
                return oh

            with rep_ctx:
                for si, st in enumerate(structure):
                    NW = st["NW"]
                    if NW == 0:
                        continue
                    idx_sb = idxp.tile([P, NW * 8], mybir.dt.int16, tag="idx")
                    nc.sync.dma_start(idx_sb[:], idx_ts[si][:])
                    fwin = feature_t[st["row0"] : st["row0"] + st["rows"], :]
                    msg = None
                    wb_cursor = 0
                    for gg, nbk in st["buckets"]:
                        gsl = slice(gg * P, (gg + 1) * P)
                        agg_ps = aggp.tile([P, D], f32)
                        carry = si != first_win_of_g[gg]
                        if carry:
                            nc.tensor.matmul(
                                out=agg_ps[:],
                                lhsT=ident_sb[:],
                                rhs=agg_acc[:, gsl],
                                start=True,
                                stop=False,
                            )
                        for j in range(nbk):
                            wb = wb_cursor + j
                            c, r = divmod(wb, CH)
                            if r == 0:
                                cw = min(CH, NW - c * CH)
                                if "gather" in ablate:
                                    msg = msg0
                                else:
                                    msg = msgp.tile([P, CH, D], dt, tag="msg")
                                    nc.gpsimd.dma_gather(
                                        out_ap=msg[:, :cw, :],
                                        in_ap=fwin,
                                        idxs_ap=idx_sb[:, c * CH * 8 : (c * CH + cw) * 8],
                                        num_idxs=cw * P,
                                        num_idxs_reg=cw * P,
                                        elem_size=D,
                                        queue_num=gq_count[0] % 4,
                                        single_packet=False,
                                    )
                                gq_count[0] += 1
                            bb = st["woff"] + wb
                            oh = make_onehot(bb)
                            nc.tensor.matmul(
                                out=agg_ps[:],
                                lhsT=oh[:],
                                rhs=msg[:, r, :],
                                start=(j == 0 and not carry),
                                stop=(j == nbk - 1),
                            )
                        wb_cursor += nbk
                        nc.scalar.activation(
                            out=agg_acc[:, gsl], in_=agg_ps[:], func=Copy
                        )
                        if last_win_of_g[gg] == si:
                            tail(gg)

                # groups with no edges at all: agg is zero -> z = 0 -> relu 0.
                # still must contribute to the reduce chain count; memset + tail.
                for gg in range(n_groups):
                    if gg not in seen_g:
                        nc.vector.memset(agg_acc[:, gg * P : (gg + 1) * P], 0.0)
                        tail(gg)

                res_sb = constp.tile([1, D], f32)
                nc.vector.tensor_copy(res_sb[:], out_acc[0:1, :])
                nc.sync.dma_start(out_t[:], res_sb[:])

    nc.compile()
    return nc


def prepare(feature, edge_src, edge_dst, edge_val, W, b, reps=1,
            use_bf16=True, CH=6):
    """Build the Bass program + per-core input maps. Returns (nc, in_maps, N)."""
    N, D = feature.shape
    E = edge_src.shape[0]
    assert D == P

    feature = np.ascontiguousarray(feature, dtype=np.float32)
    edge_src = np.asarray(edge_src, dtype=np.int64)
    edge_dst = np.asarray(edge_dst, dtype=np.int64)
    edge_val = np.asarray(edge_val, dtype=np.float32)

    plan = _plan(N, E, edge_src, edge_dst, edge_val)

    import os as _os
    ablate = tuple(_os.environ.get("ABLATE", "").split(",")) if _os.environ.get("ABLATE") else ()
    dt = mybir.dt.bfloat16 if use_bf16 else mybir.dt.float32
    nc = _build_program(N, D, plan, dt, CH=CH, reps=reps, ablate=ablate)

    npdt = mybir.dt.np(dt)
    wt = np.ascontiguousarray(W.astype(np.float32).T)
    brow = np.ascontiguousarray(b.astype(np.float32).reshape(1, D))
    iota = np.tile(np.arange(P, dtype=np.float32), (P, 1)).astype(npdt)
    ident = np.eye(P, dtype=np.float32)
    ones = np.ones((P, 1), dtype=np.float32)
    feature_c = feature.astype(npdt)

    in_maps = []
    for c in range(N_CORES):
        m = {
            "feature": feature_c,
            "ld": plan["ldv"][c],
            "val": plan["valv"][c],
            "s": plan["s"][c],
            "wt": wt,
            "brow": brow,
            "iota": iota,
            "ident": ident,
            "ones": ones,
        }
        for st in plan["structure"]:
            m[f"idxw{st['w']}"] = plan["idx"][st["w"]][c]
        in_maps.append(m)

    return nc, in_maps, N


def combine(results, N):
    parts = np.stack([results[c]["out"][0] for c in range(N_CORES)])
    return (parts.sum(axis=0, dtype=np.float64) / N).astype(np.float32)


def kernel(feature, edge_src, edge_dst, edge_val, W, b):
    nc, in_maps, N = prepare(feature, edge_src, edge_dst, edge_val, W, b)
    res = run_bass_kernel_spmd(nc, in_maps, core_ids=list(range(N_CORES)))
    kernel.last = res  # for test.py profiling; harmless in harness
    return combine(res.results, N)

